# revision 2
# baseline (speedup 1.0000x reference)
"""GATv2 state encoder on 8 Trainium2 NeuronCores (Bass/Tile) — fused single
launch.

Sharding: nodes split 8 ways by id (6250/core); each directed edge (plus self
loops) is processed by the core owning its dst. Per core, edges are grouped
into 128-node blocks and 128-edge chunks (within a block edges are split by
src half for int16 gather indices, each side padded to KA=KB=8 chunks).

Single kernel launch does both convs:
  dense1 (local 6250 nodes -> xl1/xr1 rows) -> AllGather xl1 table across the
  8 cores -> edge1 (gather xl[src]/xr[dst], attention, exp, slot-matmul
  scatter) producing h1^T tiles kept in SBUF -> dense2 from SBUF h1^T ->
  AllGather xl2 table -> edge2 -> masked pool-sum -> [1,32] per core.
Host sums the 8 pool partials, divides by N, applies the final 32->96 linear.

Per-core uploads are minimized (the axon tunnel runs ~60MB/s): local x shard
and the small weight matrices in bf16, dedup'd int16 gather indices
(replicated 16->128 partitions on device), uint8 per-edge slot ids (one-hot
scatter matrices are built on device with an iota/is_equal compare), and all
per-channel constants packed into one f32 row that is partition-broadcast on
device. Scratch tables live in internal DRAM.
"""
import numpy as np
import ml_dtypes

N = 50000
NC = 8
NSH = N // NC               # 6250
NBLK = (NSH + 127) // 128   # 49
NPAD = NBLK * 128           # 6272
HALF = 25088                # src half split (int16-safe)
KA = 8
KB = 8
KCH = KA + KB
P = 128
S1, S2, S3 = KA * P // 16, KB * P // 16, KCH * P // 16   # 64, 64, 128
SB = S1 + S2 + S3                                        # 256
CE1, CT1, H1 = 128, 128, 2
CE2, CT2, H2 = 64, 32, 1
LASTL = NSH - (NBLK - 1) * P    # 106

# packed f32 constants row: offset, length
_COM = {}
_off = 0
for _nm, _ln in (("b1A", CE1), ("b1B", CE1), ("bo1", CT1),
                 ("b2A", CE2), ("b2B", CE2), ("bo2", CT2),
                 ("at1", CE1), ("at1b", CE1), ("at2", CE2), ("at2b", CE2),
                 ("iota", P)):
    _COM[_nm] = (_off, _ln)
    _off += _ln
NCOM = _off

_cache = {}


def preprocess(edge_index):
    src = np.concatenate([np.asarray(edge_index[0], np.int64),
                          np.arange(N, dtype=np.int64)])
    dst = np.concatenate([np.asarray(edge_index[1], np.int64),
                          np.arange(N, dtype=np.int64)])
    order = np.argsort(dst, kind='stable')
    src, dst = src[order], dst[order]

    def wrap16(ix):  # [n] -> [16, n//16] int16 (16-partition wrapped)
        n = ix.shape[0]
        a = np.zeros((16, n // 16), np.int16)
        a[np.arange(n) % 16, np.arange(n) // 16] = ix.astype(np.int16)
        return a

    cores = []
    for c in range(NC):
        lo, hi = c * NSH, (c + 1) * NSH
        m = (dst >= lo) & (dst < hi)
        s, d = src[m], (dst[m] - lo)
        srcs = np.zeros((NBLK, KCH, P), np.int64)
        slot = np.full((NBLK, KCH, P), 255, np.int32)
        dstl = np.zeros((NBLK, KCH, P), np.int64)
        for b in range(NBLK):
            mm = (d >= b * 128) & (d < (b + 1) * 128)
            sb, db = s[mm], d[mm]
            amask = sb < HALF
            for side in range(2):
                ss = sb[amask] if side == 0 else sb[~amask]
                dd = db[amask] if side == 0 else db[~amask]
                k0, kmax = (0, KA) if side == 0 else (KA, KB)
                cnt = ss.shape[0]
                assert cnt <= kmax * P
                for k in range((cnt + P - 1) // P):
                    n = min(P, cnt - k * P)
                    sl = slice(k * P, k * P + n)
                    srcs[b, k0 + k, :n] = ss[sl]
                    dstl[b, k0 + k, :n] = dd[sl]
                    slot[b, k0 + k, :n] = dd[sl] - b * 128

        idx16 = np.zeros((NBLK, 16, SB), np.int16)
        for b in range(NBLK):
            va = slot[b, :KA].reshape(-1) < 128
            vb = slot[b, KA:].reshape(-1) < 128
            sa = np.where(va, srcs[b, :KA].reshape(-1), 0)
            sbb = np.where(vb, srcs[b, KA:].reshape(-1) - HALF, 0)
            dr = np.where(slot[b].reshape(-1) < 128, dstl[b].reshape(-1), 0)
            idx16[b, :, 0:S1] = wrap16(sa)
            idx16[b, :, S1:S1 + S2] = wrap16(sbb)
            idx16[b, :, S1 + S2:SB] = wrap16(dr)

        slotT = np.ascontiguousarray(
            slot.transpose(2, 0, 1).astype(np.uint8))   # [P, NBLK, KCH]
        cores.append(dict(idx16=idx16, slotT=slotT))
    return cores


def build(debug=False):
    import concourse.mybir as mybir
    import concourse.tile as tile
    import concourse.bacc as bacc

    nc = bacc.Bacc("TRN2", num_devices=NC)
    dt = mybir.dt
    f32, bf16, i16 = dt.float32, dt.bfloat16, dt.int16
    u8, i32 = dt.uint8, dt.int32
    ALU = mybir.AluOpType
    ACT = mybir.ActivationFunctionType

    d_x = nc.dram_tensor("xlocT", [P, NPAD], bf16, kind="ExternalInput")
    d_W1A = nc.dram_tensor("W1A", [P, CE1], bf16, kind="ExternalInput")
    d_W1B = nc.dram_tensor("W1B", [P, CE1], bf16, kind="ExternalInput")
    d_W2A = nc.dram_tensor("W2A", [P, CE2], bf16, kind="ExternalInput")
    d_W2B = nc.dram_tensor("W2B", [P, CE2], bf16, kind="ExternalInput")
    d_idx16 = nc.dram_tensor("idx16", [NBLK, 16, SB], i16,
                             kind="ExternalInput")
    d_slot = nc.dram_tensor("slotT", [P, NBLK, KCH], u8,
                            kind="ExternalInput")
    d_com = nc.dram_tensor("com", [1, NCOM], f32, kind="ExternalInput")

    d_pool = nc.dram_tensor("pool_out", [1, CT2], f32, kind="ExternalOutput")
    d_h1 = d_h2 = None
    if debug:
        d_h1 = nc.dram_tensor("h1_out", [NSH, CT1], f32,
                              kind="ExternalOutput")
        d_h2 = nc.dram_tensor("h2_out", [NSH, CT2], f32,
                              kind="ExternalOutput")

    # internal DRAM scratch
    d_t1A = nc.dram_tensor("t1A", [NSH, CE1], f32)
    d_tab1 = nc.dram_tensor("tab1", [N, CE1], f32, addr_space="Shared")
    d_tR1 = nc.dram_tensor("tR1", [NPAD, CE1], f32)
    d_t2A = nc.dram_tensor("t2A", [NSH, CE2], f32)
    d_tab2 = nc.dram_tensor("tab2", [N, CE2], f32, addr_space="Shared")
    d_tR2 = nc.dram_tensor("tR2", [NPAD, CE2], f32)

    with tile.TileContext(nc) as tc:
        with tc.tile_pool(name="const", bufs=1) as constp:
            def ctile(nm, shape, dtype):
                return constp.tile(shape, dtype, name=nm, tag=nm)

            def cload(nm, d, shape, dtype):
                t = ctile(nm, shape, dtype)
                nc.sync.dma_start(t[:], d[:])
                return t

            t_x = cload("cx", d_x, [P, NPAD], bf16)
            t_W1A = cload("cW1A", d_W1A, [P, CE1], bf16)
            t_W1B = cload("cW1B", d_W1B, [P, CE1], bf16)
            t_W2A = cload("cW2A", d_W2A, [P, CE2], bf16)
            t_W2B = cload("cW2B", d_W2B, [P, CE2], bf16)
            t_slotu = cload("cslotu", d_slot, [P, NBLK, KCH], u8)
            t_com0 = cload("ccom0", d_com, [1, NCOM], f32)
            t_com = ctile("ccom", [P, NCOM], f32)
            nc.gpsimd.partition_broadcast(t_com[:], t_com0[:])

            def com(nm):
                o, ln = _COM[nm]
                return t_com[:, o:o + ln]

            t_b1A, t_b1B, t_bo1 = com("b1A"), com("b1B"), com("bo1")
            t_b2A, t_b2B, t_bo2 = com("b2A"), com("b2B"), com("bo2")
            t_at1b, t_at2b = com("at1b"), com("at2b")
            t_iota = com("iota")
            t_at1 = ctile("cat1", [P, CE1], bf16)
            nc.vector.tensor_copy(t_at1[:], com("at1"))
            t_at2 = ctile("cat2", [P, CE2], bf16)
            nc.vector.tensor_copy(t_at2[:], com("at2"))

            t_slot = ctile("cslot", [P, NBLK, KCH], f32)
            nc.vector.tensor_copy(t_slot[:], t_slotu[:])

            # identity for PE transpose: eye[p, j] = (j == p)
            t_pidx = ctile("cpidx", [P, 1], i32)
            nc.gpsimd.iota(t_pidx[:], [[1, 1]], channel_multiplier=1)
            t_pidxf = ctile("cpidxf", [P, 1], f32)
            nc.vector.tensor_copy(t_pidxf[:], t_pidx[:])
            t_eye = ctile("ceye", [P, P], f32)
            nc.vector.tensor_tensor(
                out=t_eye[:], in0=t_pidxf[:].to_broadcast([P, P]),
                in1=t_iota, op=ALU.is_equal)

            # pool mask: (b*128 + p) < NSH
            t_nidx = ctile("cnidx", [P, NBLK], i32)
            nc.gpsimd.iota(t_nidx[:], [[128, NBLK]], channel_multiplier=1)
            t_nidxf = ctile("cnidxf", [P, NBLK], f32)
            nc.vector.tensor_copy(t_nidxf[:], t_nidx[:])
            t_pm = ctile("cpm", [P, NBLK], f32)
            nc.vector.tensor_scalar(out=t_pm[:], in0=t_nidxf[:],
                                    scalar1=float(NSH), scalar2=None,
                                    op0=ALU.is_lt)

            t_h1T = ctile("ch1T", [P, NPAD], bf16)

            # replicate gather indices 16 -> 128 partitions (SBUF-resident)
            t_idx = ctile("cidx", [P, NBLK, SB], i16)
            for k in range(8):
                nc.sync.dma_start(
                    t_idx[16 * k:16 * (k + 1), :, :],
                    d_idx16[:, :, :].rearrange("b p s -> p b s"))

            def dense(t_src, t_WA, t_WB, t_bA, t_bB, dA, dR, CE):
                with (
                    tc.tile_pool(name="dense", bufs=3) as dp,
                    tc.tile_pool(name="dps", bufs=4, space="PSUM") as dps,
                ):
                    for j in range(NBLK):
                        m = P if j < NBLK - 1 else LASTL
                        ps = dps.tile([P, CE], f32, tag="ps")
                        nc.tensor.matmul(ps[0:m, :],
                                         lhsT=t_src[:, j * P:j * P + m],
                                         rhs=t_WA[:], start=True, stop=True)
                        to = dp.tile([P, CE], f32, tag="toA")
                        nc.vector.tensor_tensor(out=to[0:m, :], in0=ps[0:m, :],
                                                in1=t_bA[0:m], op=ALU.add)
                        nc.sync.dma_start(dA[j * P:j * P + m, :], to[0:m, :])
                        ps2 = dps.tile([P, CE], f32, tag="ps")
                        nc.tensor.matmul(ps2[0:m, :],
                                         lhsT=t_src[:, j * P:j * P + m],
                                         rhs=t_WB[:], start=True, stop=True)
                        to2 = dp.tile([P, CE], f32, tag="toB")
                        nc.vector.tensor_tensor(out=to2[0:m, :],
                                                in0=ps2[0:m, :],
                                                in1=t_bB[0:m], op=ALU.add)
                        nc.sync.dma_start(dR[j * P:j * P + m, :], to2[0:m, :])

            def edge_conv(CE, CT, H, d_tab, d_tR, t_at, t_atb, t_bo,
                          first, d_hdbg):
                CEH = CE // H
                CH = CT // H
                with (
                    tc.tile_pool(name="gat", bufs=2) as gat,
                    tc.tile_pool(name="gsm", bufs=2) as gsm,
                    tc.tile_pool(name="eps", bufs=2, space="PSUM") as eps,
                    tc.tile_pool(name="pps", bufs=1, space="PSUM") as pps,
                ):
                    t_pool = None
                    if not first:
                        t_pool = pps.tile([1, CT], f32)
                    for b in range(NBLK):
                        t_idxb = t_idx[:, b, :]
                        t_xl = gat.tile([P, KCH, CE], f32, tag="xl")
                        nc.gpsimd.dma_gather(
                            out_ap=t_xl[:, 0:KA, :], in_ap=d_tab[0:HALF, :],
                            idxs_ap=t_idxb[:, 0:S1],
                            num_idxs=KA * P, num_idxs_reg=KA * P,
                            elem_size=CE)
                        nc.gpsimd.dma_gather(
                            out_ap=t_xl[:, KA:KCH, :], in_ap=d_tab[HALF:N, :],
                            idxs_ap=t_idxb[:, S1:S1 + S2],
                            num_idxs=KB * P, num_idxs_reg=KB * P,
                            elem_size=CE)
                        t_xr = gat.tile([P, KCH, CE], f32, tag="xr")
                        half3 = S3 // 2
                        nc.gpsimd.dma_gather(
                            out_ap=t_xr[:, 0:KCH // 2, :], in_ap=d_tR[:],
                            idxs_ap=t_idxb[:, S1 + S2:S1 + S2 + half3],
                            num_idxs=KCH * P // 2, num_idxs_reg=KCH * P // 2,
                            elem_size=CE)
                        nc.gpsimd.dma_gather(
                            out_ap=t_xr[:, KCH // 2:KCH, :], in_ap=d_tR[:],
                            idxs_ap=t_idxb[:, S1 + S2 + half3:SB],
                            num_idxs=KCH * P // 2, num_idxs_reg=KCH * P // 2,
                            elem_size=CE)

                        # one-hot scatter matrices from slot ids
                        t_ms = gsm.tile([P, KCH, P], bf16, tag="ms")
                        nc.vector.tensor_tensor(
                            out=t_ms[:],
                            in0=t_slot[:, b, :].unsqueeze(2).to_broadcast(
                                [P, KCH, P]),
                            in1=t_iota.unsqueeze(1).to_broadcast(
                                [P, KCH, P]),
                            op=ALU.is_equal)

                        t_z = gat.tile([P, KCH, CE], f32, tag="z")
                        nc.vector.tensor_tensor(out=t_z[:], in0=t_xl[:],
                                                in1=t_xr[:], op=ALU.add)
                        t_zp = gsm.tile([P, KCH, CE], bf16, tag="zp")
                        nc.scalar.activation(t_zp[:], t_z[:], ACT.Relu)
                        # lrelu(z).att = (0.8 att).relu(z) + (0.2 att).z
                        t_am = gsm.tile([P, KCH, 2, CE], bf16, tag="am")
                        attb = t_at[:].unsqueeze(1).to_broadcast([P, KCH, CE])
                        nc.vector.tensor_tensor(out=t_am[:, :, 0, :],
                                                in0=t_zp[:], in1=attb,
                                                op=ALU.mult)
                        att2b = t_atb.unsqueeze(1).to_broadcast(
                            [P, KCH, CE])
                        nc.vector.tensor_tensor(out=t_am[:, :, 1, :],
                                                in0=t_z[:], in1=att2b,
                                                op=ALU.mult)
                        t_red = gsm.tile([P, KCH, H], f32, tag="red")
                        am_g = t_am[:].rearrange("p k s (h c) -> p k h s c",
                                                 h=H)
                        nc.vector.tensor_reduce(out=t_red[:], in_=am_g,
                                                axis=mybir.AxisListType.XY,
                                                op=ALU.add)
                        t_ex = gsm.tile([P, KCH, H], f32, tag="ex")
                        nc.scalar.activation(t_ex[:], t_red[:], ACT.Exp)
                        t_pay = gsm.tile([P, KCH, CE + H], bf16, tag="pay")
                        ex_b = t_ex[:].unsqueeze(3).to_broadcast(
                            [P, KCH, H, CEH])
                        pay4 = t_pay[:, :, 0:CE].rearrange(
                            "p k (h c) -> p k h c", h=H)
                        xl4 = t_xl[:].rearrange("p k (h c) -> p k h c", h=H)
                        nc.vector.tensor_tensor(out=pay4, in0=xl4, in1=ex_b,
                                                op=ALU.mult)
                        nc.vector.tensor_copy(t_pay[:, :, CE:CE + H], t_ex[:])

                        t_seg = eps.tile([P, CE + H], f32, tag="seg")
                        for k in range(KCH):
                            nc.tensor.matmul(t_seg[:], lhsT=t_ms[:, k, :],
                                             rhs=t_pay[:, k, :],
                                             start=(k == 0),
                                             stop=(k == KCH - 1))

                        t_s = gsm.tile([P, H], f32, tag="s")
                        nc.vector.tensor_scalar(out=t_s[:],
                                                in0=t_seg[:, CE:CE + H],
                                                scalar1=1e-30, scalar2=None,
                                                op0=ALU.max)
                        t_rec = gsm.tile([P, H], f32, tag="rec")
                        nc.vector.reciprocal(t_rec[:], t_s[:])
                        t_hn = gsm.tile([P, CT], f32, tag="hn")
                        rec_b = t_rec[:].unsqueeze(2).to_broadcast([P, H, CH])
                        hn3 = t_hn[:].rearrange("p (h c) -> p h c", h=H)
                        seg3 = t_seg[:, 0:CE].rearrange("p (h c) -> p h c",
                                                        h=H)
                        nc.vector.tensor_tensor(out=hn3,
                                                in0=seg3[:, :, 0:CH],
                                                in1=rec_b, op=ALU.mult)
                        t_hb = gsm.tile([P, CT], f32, tag="hb")
                        nc.vector.tensor_tensor(out=t_hb[:], in0=t_hn[:],
                                                in1=t_bo[0:P],
                                                op=ALU.add)
                        t_h = gsm.tile([P, CT], f32, tag="h")
                        nc.scalar.activation(t_h[:], t_hb[:], ACT.Relu)
                        if d_hdbg is not None:
                            m = P if b < NBLK - 1 else LASTL
                            nc.sync.dma_start(d_hdbg[b * P:b * P + m, :],
                                              t_h[0:m, :])
                        if first:
                            ps_t = eps.tile([P, P], f32, tag="tr")
                            nc.tensor.transpose(ps_t[:], t_h[:], t_eye[:])
                            nc.scalar.copy(t_h1T[:, b * P:(b + 1) * P],
                                           ps_t[:])
                        else:
                            nc.tensor.matmul(t_pool[:],
                                             lhsT=t_pm[:, b:b + 1],
                                             rhs=t_h[:],
                                             start=(b == 0),
                                             stop=(b == NBLK - 1))
                    if not first:
                        t_po = gsm.tile([1, CT], f32, tag="po")
                        nc.vector.tensor_copy(t_po[:], t_pool[:])
                        nc.sync.dma_start(d_pool[:], t_po[:])

            RG = [list(range(NC))]
            BYP = ALU.bypass

            dense(t_x, t_W1A, t_W1B, t_b1A, t_b1B, d_t1A, d_tR1, CE1)
            nc.gpsimd.collective_compute(
                "AllGather", BYP, replica_groups=RG,
                ins=[d_t1A[:, :].opt()], outs=[d_tab1[:, :].opt()])
            edge_conv(CE1, CT1, H1, d_tab1, d_tR1, t_at1, t_at1b, t_bo1,
                      True, d_h1)
            dense(t_h1T, t_W2A, t_W2B, t_b2A, t_b2B, d_t2A, d_tR2, CE2)
            nc.gpsimd.collective_compute(
                "AllGather", BYP, replica_groups=RG,
                ins=[d_t2A[:, :].opt()], outs=[d_tab2[:, :].opt()])
            edge_conv(CE2, CT2, H2, d_tab2, d_tR2, t_at2, t_at2b, t_bo2,
                      False, d_h2)

    nc.compile()
    return nc


def _attr_row(att, CT, CE, H, scale):
    CH = CT // H
    a = np.zeros((CE,), np.float32)
    for h in range(H):
        a[h * (CE // H):h * (CE // H) + CH] = scale * att.reshape(H, CH)[h]
    return a


def _padv(v, ln):
    out = np.zeros((ln,), np.float32)
    out[:v.shape[0]] = v
    return out


def _padT16(W, cols):  # W.T padded to [P, cols], bf16
    out = np.zeros((P, cols), ml_dtypes.bfloat16)
    WT = np.ascontiguousarray(W.T)
    out[:WT.shape[0], :WT.shape[1]] = WT.astype(ml_dtypes.bfloat16)
    return out


def _run(nc, maps):
    import os, time
    from concourse import bass_utils
    trace = bool(int(os.environ.get("GAT_TRACE", "0")))
    t0 = time.time()
    r = bass_utils.run_bass_kernel_spmd(nc, maps, core_ids=list(range(NC)),
                                        trace=trace)
    _cache.setdefault('run_wall', []).append(time.time() - t0)
    if getattr(r, 'exec_time_ns', None):
        _cache.setdefault('exec_ns', []).append(r.exec_time_ns)
    return r


def kernel(x, edge_index, batch, Win, b_in, Wl1, bl1, Wr1, br1, att1, bias1,
           Wl2, bl2, Wr2, br2, att2, bias2, Wout, b_out):
    import os
    debug = bool(int(os.environ.get("GAT_DEBUG", "0")))
    x = np.asarray(x, np.float32)
    edge_index = np.asarray(edge_index)
    Win, b_in = np.asarray(Win, np.float32), np.asarray(b_in, np.float32)
    Wl1, bl1 = np.asarray(Wl1, np.float32), np.asarray(bl1, np.float32)
    Wr1, br1 = np.asarray(Wr1, np.float32), np.asarray(br1, np.float32)
    att1 = np.asarray(att1, np.float32)
    bias1 = np.asarray(bias1, np.float32)
    Wl2, bl2 = np.asarray(Wl2, np.float32), np.asarray(bl2, np.float32)
    Wr2, br2 = np.asarray(Wr2, np.float32), np.asarray(br2, np.float32)
    att2 = np.asarray(att2, np.float32)
    bias2 = np.asarray(bias2, np.float32)
    Wout, b_out = np.asarray(Wout, np.float32), np.asarray(b_out, np.float32)

    pre = _cache.get('pre')
    if pre is None or not np.array_equal(_cache.get('ei'), edge_index):
        pre = preprocess(edge_index)
        _cache['pre'] = pre
        _cache['ei'] = np.asarray(edge_index).copy()

    WA1, bA1 = Wl1 @ Win, Wl1 @ b_in + bl1
    WB1, bB1 = Wr1 @ Win, Wr1 @ b_in + br1

    key = ('nc', debug)
    if key not in _cache:
        _cache[key] = build(debug)

    com = np.zeros((1, NCOM), np.float32)

    def setcom(nm, v):
        o, ln = _COM[nm]
        com[0, o:o + ln] = _padv(v, ln)

    setcom("b1A", bA1)
    setcom("b1B", bB1)
    setcom("bo1", bias1)
    setcom("b2A", bl2)
    setcom("b2B", br2)
    setcom("bo2", bias2)
    setcom("at1", _attr_row(att1, CT1, CE1, H1, 0.8))
    setcom("at1b", _attr_row(att1, CT1, CE1, H1, 0.2))
    setcom("at2", _attr_row(att2, CT2, CE2, H2, 0.8))
    setcom("at2b", _attr_row(att2, CT2, CE2, H2, 0.2))
    setcom("iota", np.arange(P, dtype=np.float32))

    common = {
        "W1A": _padT16(WA1, CE1), "W1B": _padT16(WB1, CE1),
        "W2A": _padT16(Wl2, CE2), "W2B": _padT16(Wr2, CE2),
        "com": com,
    }
    maps = []
    for c in range(NC):
        xloc = np.zeros((P, NPAD), ml_dtypes.bfloat16)
        xloc[:, :NSH] = x[c * NSH:(c + 1) * NSH].T.astype(ml_dtypes.bfloat16)
        m = dict(common)
        m["xlocT"] = xloc
        m["idx16"] = pre[c]['idx16']
        m["slotT"] = pre[c]['slotT']
        maps.append(m)

    res = _run(_cache[key], maps)
    if debug:
        _cache['h1'] = np.concatenate(
            [np.asarray(res.results[c]["h1_out"], np.float32)
             for c in range(NC)], 0)
        _cache['h2'] = np.concatenate(
            [np.asarray(res.results[c]["h2_out"], np.float32)
             for c in range(NC)], 0)
    pooled = sum(np.asarray(res.results[c]["pool_out"], np.float32)
                 for c in range(NC)).reshape(CT2)
    pooled = pooled / np.float32(N)
    out = pooled @ Wout.T + b_out
    return out[None, :].astype(np.float32)
